# revision 27
# baseline (speedup 1.0000x reference)
"""Trainium2 Bass kernel for additive (Bahdanau) attention.

    c[b] = softmax_t( v_a . tanh(s[b] @ W_a + h[b] @ U_a) ) @ h[b]

Shapes (hardcoded): s [32,1024] f32, h [32,2048,1024] f32,
W_a [1024,512], U_a [1024,512], v_a [512]  ->  c [32,1024] f32.

Sharding: data-parallel over batch; 8 NeuronCores x 4 batches each.
W_a/U_a/v_a replicated. No cross-core communication.

v3 design (evolved from trace analysis of v1 at 246.8us / v2 at 203.7us):
  - PE is the bottleneck engine; its per-supertile work is 32 main
    matmuls (N=512) + 32 transposes (N=128) + 4 v-dot matmuls ~ 9.6us.
    Everything else must stay off its critical path and the HAM clock
    must stay at 2.4 GHz (PE-idle or transpose-only stretches > ~3.4us
    re-throttle it to 1.2 GHz).
  - make_identity runs before any dma_start (gpsimd queue is FIFO; in
    v2 the identity landed behind 9 DMA triggers and the PE sat idle
    for 16us). Dummy warm-up matmuls engage the HAM fast clock while
    the first DMAs stream in.
  - The v-dot uses a replicated stationary (v (x) ones_row, M=128):
    the logit row lands replicated on all 128 PSUM partitions for the
    same column count, making exp / row-sum / 1/S / the p-weighted
    c-reduction all partition-local.  c runs on the DVE as
    scalar_tensor_tensor(hT[dc] * p_exp -> accum) against the hT tiles
    already in SBUF (tensor_tensor_reduce hard-faults trn2 hardware;
    scalar_tensor_tensor with accum_out is the working equivalent).
  - The vdot/exp/c chain for supertile g runs one iteration deferred
    (during mains of g+1), so ACT/DVE latency never gates the PE.
  - hT PSUM->SBUF copies split 2 groups DVE / 2 groups ScalarE to
    balance engine load (DVE also carries the 8 c-reductions).
"""

import numpy as np

import concourse.bacc as bacc
import concourse.tile as tile
import concourse.mybir as mybir
from concourse.bass_utils import run_bass_kernel_spmd
from concourse.masks import make_identity

N_CORES = 8
B, T, DH, DS, A = 32, 2048, 1024, 1024, 512
BPC = B // N_CORES          # batches per core
ST = 512                    # supertile rows (t)
NST = T // ST               # supertiles per batch
NTS = ST // 128             # 128-row chunks per supertile
NDC = DH // 128             # d chunks
NAC = A // 128              # a chunks
NGLOB = BPC * NST           # supertiles per core

F32 = mybir.dt.float32
BF16 = mybir.dt.bfloat16
FP8 = mybir.dt.float8e4
DR = mybir.MatmulPerfMode.DoubleRow
AF = mybir.ActivationFunctionType
MUL = mybir.AluOpType.mult

# number of d-chunk PAIRS of the main matmul done in fp8 DoubleRow mode
# (2x PE throughput). 0 = all bf16; 2 = chunks 0-3 fp8; 4 = all fp8.
N_FP8_PAIRS = 4
# trailing d-chunks whose c-partial runs as narrow PE matmuls against
# h natural (frees the DVE stt + the bf16 hT copy for those chunks)
C_PE = 2
N_STT = 8 - C_PE  # leading chunks on the DVE stt path
I32 = mybir.dt.int32


def build_nc():
    assert 2 * N_FP8_PAIRS == NDC, "only the full-fp8 mains path is wired"
    nc = bacc.Bacc("TRN2", target_bir_lowering=False, debug=False,
                   num_devices=N_CORES)
    s = nc.dram_tensor("s", [BPC, DS], F32, kind="ExternalInput").ap()
    h = nc.dram_tensor("h", [BPC, T, DH], F32, kind="ExternalInput").ap()
    W_a = nc.dram_tensor("W_a", [DS, A], F32, kind="ExternalInput").ap()
    U_a = nc.dram_tensor("U_a", [DH, A], F32, kind="ExternalInput").ap()
    v_a = nc.dram_tensor("v_a", [A], F32, kind="ExternalInput").ap()
    c = nc.dram_tensor("c", [BPC, DH], F32, kind="ExternalOutput").ap()

    with tile.TileContext(nc) as tc:
        with (
            tc.tile_pool(name="const", bufs=1) as const,
            tc.tile_pool(name="hpool", bufs=4) as hpool,
            tc.tile_pool(name="ht8pool", bufs=4) as ht8pool,
            tc.tile_pool(name="esbp", bufs=12) as esbp,
            tc.tile_pool(name="pexpp", bufs=3) as pexpp,
            tc.tile_pool(name="hppool", bufs=3) as hppool,
            tc.tile_pool(name="smalls", bufs=4) as smalls,
            tc.tile_pool(name="cres", bufs=3) as cres,
            tc.tile_pool(name="epool", bufs=4, space="PSUM") as epool,
            tc.tile_pool(name="tpsp", bufs=2, space="PSUM") as tpsp,
            tc.tile_pool(name="prp", bufs=2, space="PSUM") as prp,
        ):
            # ---- identity/ones BEFORE any dma_start: the gpsimd queue
            # is FIFO and the PE warm-up depends on the identity.
            ident = const.tile([128, 128], BF16, name="ident")
            make_identity(nc, ident)
            ones_row = const.tile([1, 128], BF16)
            nc.vector.memset(ones_row, 1.0)
            ones32 = const.tile([128, 32], BF16, name="ones32")
            nc.vector.memset(ones32, 1.0)

            h_tiles = {}
            ht8_tiles = {}
            P8 = N_FP8_PAIRS

            warm_ps_holder = []

            def dummy_mms(n):
                for i in range(n):
                    nc.tensor.matmul(warm_ps_holder[0][:, 0:128], lhsT=ident,
                                     rhs=ident, start=True, stop=True,
                                     skip_group_check=True)

            def load_h(glob):
                # quartered: per-ts DMAs so transposes unblock per-chunk
                b, st = glob // NST, glob % NST
                t = hpool.tile([128, NTS, DH], BF16, name=f"h_sb{glob}",
                               tag="h_sb")
                for ts in range(NTS):
                    nc.gpsimd.dma_start(
                        out=t[:, ts],
                        in_=h[b, ST * st + 128 * ts:
                              ST * st + 128 * (ts + 1), :])
                h_tiles[glob] = t

            def xbar_h(glob):
                # PE transposes 32 [128,128] chunks -> bf16 PSUM.
                # bf16 hT kept only for the N_STT stt chunks, copied as
                # int32-bitcast on DVE (half the element count, bit-exact).
                # fp8 hT8 (all chunks, DoubleRow mains): groups 0,1 cast on
                # ScalarE, groups 2,3 via SWDGE casting DMA.
                h_sb = h_tiles[glob]
                ht8 = ht8pool.tile([128, NDC, NTS, 128], FP8,
                                   name=f"hT8_sb{glob}", tag="hT8_sb")
                for dcp in range(NDC // 2):
                    tps = tpsp.tile([128, 1024], BF16,
                                    name=f"tps{glob}_{dcp}", tag="tps")
                    for dch in range(2):
                        dc = 2 * dcp + dch
                        for ts in range(NTS):
                            nc.tensor.transpose(
                                tps[:, dch * 512 + ts * 128:
                                    dch * 512 + ts * 128 + 128],
                                h_sb[:, ts, 128 * dc:128 * (dc + 1)],
                                ident)
                    tview = tps.rearrange("p (dch ts t) -> p dch ts t",
                                          dch=2, ts=NTS)
                    # fp8 hT8 casts from PSUM: groups 0,1 on ScalarE,
                    # groups 2,3 on DVE
                    if dcp < 2:
                        nc.scalar.copy(ht8[:, 2 * dcp:2 * dcp + 2], tview)
                    else:
                        nc.vector.tensor_copy(ht8[:, 2 * dcp:2 * dcp + 2],
                                              tview)
                ht8_tiles[glob] = ht8

            # ---- DMA order: tiny consts, h0 (transposes dep), U (mains
            # dep, ~9us in), W (bias dep, ~1 supertile in), h1.
            s_sb = const.tile([BPC, DS], BF16)
            nc.gpsimd.dma_start(out=s_sb, in_=s)
            v_row = const.tile([1, A], BF16)
            nc.gpsimd.dma_start(out=v_row,
                                in_=v_a.rearrange("(o a) -> o a", o=1))
            load_h(0)
            # h1 ahead of U/W on the cast pipe: mains(0) is gated by the
            # h0 ts-copies anyway, U arrives per-ac just-in-time, and
            # xbar(1) then never stalls (the v7 2.5us gap + HAM
            # re-throttle at ~35us).
            load_h(1)
            # U split per-ac so mains(0,ac) start as chunks land.
            # fp8 copy (DoubleRow chunks 0..2*P8-1) first - mains(0) needs
            # it before the bf16 chunks.
            if P8:
                U8_sb = const.tile([128, 2 * P8, A], FP8)
                for ac in range(NAC):
                    nc.gpsimd.dma_start(
                        out=U8_sb[:, :, 128 * ac:128 * (ac + 1)],
                        in_=U_a[0:256 * P8, 128 * ac:128 * (ac + 1)]
                        .rearrange("(dc p) a -> p dc a", p=128))
            U_sb = None
            if 2 * P8 < NDC:
                U_sb = const.tile([128, NDC - 2 * P8, A], BF16)
                for ac in range(NAC):
                    nc.gpsimd.dma_start(
                        out=U_sb[:, :, 128 * ac:128 * (ac + 1)],
                        in_=U_a[256 * P8:, 128 * ac:128 * (ac + 1)]
                        .rearrange("(dc p) a -> p dc a", p=128))
            W_sb = const.tile([128, NDC, A], BF16)
            nc.gpsimd.dma_start(out=W_sb,
                                in_=W_a.rearrange("(dc p) a -> p dc a", p=128))

            # ---- PE warm-up (dummy matmuls, results unused): keeps the
            # PE busy while U/h0 land and engages the HAM fast clock.
            warm_ps = prp.tile([128, ST], F32, name="warm_ps", tag="prp")
            warm_ps_holder.append(warm_ps)
            dummy_mms(28)

            # ---- sT via PE transpose: [128 d_lo, dc, b] bf16
            sps = epool.tile([128, NDC, BPC], BF16, name="sps", tag="e_ps",
                             padded_shape=[128, NDC, 128])
            for dc in range(NDC):
                nc.tensor.transpose(
                    sps[:, dc, :],
                    s_sb[:, 128 * dc:128 * (dc + 1)],
                    ident[0:BPC, 0:BPC])
            sT_sb = const.tile([128, NDC, BPC], BF16)
            nc.vector.tensor_copy(sT_sb, sps)

            # ---- v_rep[a_lo, ac, j] = v[a] for all j (replicated cols)
            vr_ps = prp.tile([128, ST], F32, name="vr_ps", tag="prp")
            for ac in range(NAC):
                nc.tensor.matmul(vr_ps[:, 128 * ac:128 * (ac + 1)],
                                 lhsT=v_row[:, 128 * ac:128 * (ac + 1)],
                                 rhs=ones_row, start=True, stop=True,
                                 skip_group_check=True)
            v_rep = const.tile([128, NAC, 128], BF16)
            nc.vector.tensor_copy(v_rep, vr_ps)

            bias_sb = const.tile([128, NAC, BPC], F32)

            def emit_bias():
                for ac in range(NAC):
                    ws_ps = prp.tile([128, BPC], F32, name=f"ws_ps{ac}",
                                     tag="prp", padded_shape=[128, 512])
                    for dc in range(NDC):
                        nc.tensor.matmul(
                            ws_ps,
                            lhsT=W_sb[:, dc, 128 * ac:128 * (ac + 1)],
                            rhs=sT_sb[:, dc, :],
                            start=(dc == 0), stop=(dc == NDC - 1))
                    nc.vector.tensor_copy(bias_sb[:, ac, :], ws_ps)

            # ---- first supertile's transposes, ts-major: each arriving
            # h0 quarter unblocks 8 transposes (the dcp-major order needs
            # ALL quarters for every group, so the PE would trickle 2
            # transposes per quarter and the HAM re-throttles).  Dummy
            # matmuls pad the quarter gaps to hold the fast clock.
            h0_sb = h_tiles[0]
            ht8_0 = ht8pool.tile([128, NDC, NTS, 128], FP8,
                                 name="hT8_sb0", tag="hT8_sb")
            for ts in range(NTS):
                tq = epool.tile([128, NDC, 128], BF16, name=f"tq{ts}",
                                tag="e_ps")
                for dc in range(NDC):
                    nc.tensor.transpose(tq[:, dc, :],
                                        h0_sb[:, ts, 128 * dc:128 * (dc + 1)],
                                        ident)
                nc.scalar.copy(ht8_0[:, :, ts, :], tq)
                dummy_mms(20)
            ht8_tiles[0] = ht8_0

            # ---- main loop, one-iteration-deferred softmax/c chain ----
            e_tiles = {}    # glob -> list of 4 tanh'd e_sb tiles
            S4_tiles = {}
            cpart_tiles = {}

            def deferred_stage(g):
                # v-dots (PE), exp (ACT), c-partials (DVE hp chain + PE
                # ones-contract) for supertile g.
                b, st = g // NST, g % NST
                if st == 0:
                    S4_tiles[b] = smalls.tile([128, NST], F32,
                                              name=f"S4_{b}", tag="S4")
                    # st-level c partials [32 rep, st, d] bf16
                    cpart_tiles[b] = cres.tile([32, NST, DH], BF16,
                                               name=f"cpart{b}", tag="cpart")
                e_sbs = e_tiles.pop(g)
                p_ps = prp.tile([128, ST], F32, name=f"p_ps{g}", tag="prp")
                for ac in range(NAC):
                    nc.tensor.matmul(p_ps, lhsT=v_rep[:, ac, :],
                                     rhs=e_sbs[ac],
                                     start=(ac == 0), stop=(ac == NAC - 1))
                p_exp = pexpp.tile([128, NTS, 128], BF16,
                                   name=f"p_exp{g}", tag="p_exp")
                nc.scalar.activation(p_exp, p_ps, AF.Exp,
                                     accum_out=S4_tiles[b][:, st:st + 1])
                h_nat = h_tiles.pop(g)
                # pT: p with t on partitions, via tiny PE matmuls
                pT_ps = prp.tile([128, NTS], F32, name=f"pT_ps{g}",
                                 tag="prp", padded_shape=[128, 512])
                for ts in range(NTS):
                    nc.tensor.matmul(pT_ps[:, ts:ts + 1],
                                     lhsT=p_exp[0:1, ts, :],
                                     rhs=ones_row[:, 0:1],
                                     start=True, stop=True,
                                     skip_group_check=True)
                pT_sb = smalls.tile([128, NTS], F32, name=f"pT_sb{g}",
                                    tag="pT_sb")
                nc.vector.tensor_copy(pT_sb, pT_ps)
                # hp[t_lo, d] partial sums over the 4 t-chunks: per-
                # partition scalar muls + stt adds (all bf16, 2x/4x DVE)
                hp_a = hppool.tile([128, DH], BF16, name=f"hp_a{g}",
                                   tag="hp_a")
                nc.vector.tensor_scalar_mul(hp_a, h_nat[:, 0, :],
                                            pT_sb[:, 0:1])
                nc.vector.scalar_tensor_tensor(
                    out=hp_a, in0=h_nat[:, 1, :], scalar=pT_sb[:, 1:2],
                    in1=hp_a, op0=MUL, op1=mybir.AluOpType.add)
                hp_b = hppool.tile([128, DH], BF16, name=f"hp_b{g}",
                                   tag="hp_b")
                nc.vector.tensor_scalar_mul(hp_b, h_nat[:, 2, :],
                                            pT_sb[:, 2:3])
                nc.vector.scalar_tensor_tensor(
                    out=hp_b, in0=h_nat[:, 3, :], scalar=pT_sb[:, 3:4],
                    in1=hp_b, op0=MUL, op1=mybir.AluOpType.add)
                nc.vector.tensor_add(hp_a, hp_a, hp_b)
                # PE contracts the 128 t_lo partitions with a ones block;
                # c_st replicated on 32 partitions
                c_ps = epool.tile([128, ST], F32, name=f"c_ps{g}",
                                  tag="e_ps")
                for half in range(2):
                    nc.tensor.matmul(c_ps[32 * half:32 * half + 32, :],
                                     lhsT=ones32, rhs=hp_a[:, 512 * half:
                                                          512 * (half + 1)],
                                     start=True, stop=True,
                                     skip_group_check=True)
                nc.vector.tensor_copy(cpart_tiles[b][:, st, 0:512],
                                      c_ps[0:32, :])
                nc.vector.tensor_copy(cpart_tiles[b][:, st, 512:1024],
                                      c_ps[32:64, :])
                if st == NST - 1:
                    batch_epilogue(b)

            def batch_epilogue(b):
                Ssum = smalls.tile([128, 1], F32, name=f"Ssum{b}", tag="Ssum")
                nc.vector.reduce_sum(Ssum, S4_tiles[b],
                                     axis=mybir.AxisListType.X)
                rS = smalls.tile([128, 1], F32, name=f"rS{b}", tag="rS")
                nc.vector.reciprocal(rS, Ssum)
                cp = cpart_tiles[b]
                s01 = cres.tile([32, DH], BF16, name=f"s01_{b}", tag="s01")
                nc.vector.tensor_add(s01, cp[:, 0, :], cp[:, 1, :])
                s23 = cres.tile([32, DH], BF16, name=f"s23_{b}", tag="s23")
                nc.vector.tensor_add(s23, cp[:, 2, :], cp[:, 3, :])
                csum = cres.tile([32, DH], F32, name=f"csum{b}", tag="csum")
                nc.vector.tensor_add(csum, s01, s23)
                c_fin = cres.tile([32, DH], F32, name=f"c_fin{b}",
                                  tag="c_fin")
                nc.vector.tensor_scalar_mul(c_fin, csum, rS[0:32])
                nc.sync.dma_start(out=c[b:b + 1, :], in_=c_fin[0:1, :])

            dummy_mms(24)

            for g in range(NGLOB + 1):
                if g >= 1:
                    deferred_stage(g - 1)
                if g >= NGLOB:
                    continue
                b = g // NST
                hT8 = ht8_tiles.pop(g, None)
                e_sbs = []
                e_pss = []
                for ac in range(NAC):
                    e_ps = epool.tile([128, ST], F32,
                                      name=f"e_ps{g}_{ac}", tag="e_ps")
                    for p in range(P8):
                        nc.tensor.matmul(
                            e_ps,
                            lhsT=U8_sb[:, 2 * p:2 * p + 2,
                                       128 * ac:128 * (ac + 1)],
                            rhs=hT8[:, 2 * p:2 * p + 2],
                            start=(p == 0),
                            stop=(2 * P8 == NDC and p == P8 - 1),
                            perf_mode=DR, skip_group_check=True)
                    e_pss.append(e_ps)
                    if g == 0:
                        # tanh(0,*) is emitted after emit_bias() below so
                        # the bias RAW dep is tracked; the bias matmuls
                        # then sit after mains(0) in the PE queue, where
                        # W_a has landed.
                        continue
                    e_sb = esbp.tile([128, ST], BF16,
                                     name=f"e_sb{g}_{ac}", tag="e_sb")
                    nc.scalar.activation(e_sb, e_ps, AF.Tanh,
                                         bias=bias_sb[:, ac, b:b + 1])
                    e_sbs.append(e_sb)
                    if ac == 2 and g + 2 < NGLOB:
                        load_h(g + 2)
                if g == 0:
                    emit_bias()
                    load_h(2)
                    for ac in range(NAC):
                        e_sb = esbp.tile([128, ST], BF16,
                                         name=f"e_sb0_{ac}", tag="e_sb")
                        nc.scalar.activation(e_sb, e_pss[ac], AF.Tanh,
                                             bias=bias_sb[:, ac, 0:1])
                        e_sbs.append(e_sb)
                e_tiles[g] = e_sbs
                if g + 1 < NGLOB:
                    xbar_h(g + 1)

    nc.finalize()
    return nc


_NC_CACHE = None


def kernel(s, h, W_a, U_a, v_a):
    global _NC_CACHE
    if _NC_CACHE is None:
        _NC_CACHE = build_nc()
    nc = _NC_CACHE
    s = np.ascontiguousarray(s, dtype=np.float32)
    h = np.ascontiguousarray(h, dtype=np.float32)
    W_a = np.ascontiguousarray(W_a, dtype=np.float32)
    U_a = np.ascontiguousarray(U_a, dtype=np.float32)
    v_a = np.ascontiguousarray(v_a, dtype=np.float32)
    in_maps = [
        {"s": s[i * BPC:(i + 1) * BPC], "h": h[i * BPC:(i + 1) * BPC],
         "W_a": W_a, "U_a": U_a, "v_a": v_a}
        for i in range(N_CORES)
    ]
    res = run_bass_kernel_spmd(nc, in_maps, core_ids=list(range(N_CORES)))
    return np.concatenate([res.results[i]["c"] for i in range(N_CORES)], axis=0)

